# revision 31
# baseline (speedup 1.0000x reference)
"""Causal single-head attention (shared-weight multi-head), 8-core Trainium2 Bass kernel.

Problem: embedded [4, 4096, 1024] f32, Wq/Wk/Wv [1024, 64] f32.
  q/k/v = embedded @ W*;  S = q k^T / 8 (causal);  P = softmax(S);  head = P v
  output = tile(head, 16) -> [4, 4096, 1024] f32.

Sharding: 8 cores = 4 batches x 2 parities. Core (b, r) owns q-rows
j*512 + 2*f + r for f in 0..255, j in 0..7 (every other row of every
512-block). Both cores of a batch then have IDENTICAL causal structure
(q-block j sees k-blocks 0..j) -- no padding, no role asymmetry. The host
pair-swaps x^T columns for parity-1 cores so "own" rows always sit at even
columns; the per-core triangular mask encodes the true k positions.

Inputs: x^T ships as fp8e4 except block 0, which stays bf16 -- rows with
few softmax candidates can't average out fp8 noise, every other row can
(measured). Weights stay bf16; mixed bf16 x fp8 matmuls are exact on HW.

Per q-block j: Q^T [64->128, 256] (own rows), K^T [64->128, 512] and
V [512, 65] per k-block (the 65th V column is ones so PV accumulates the
softmax denominator for free). Q^T/K^T are zero-padded to 128 contraction
rows (upper half zeroed once by the idle GPSIMD engine): K=128 matmuls
stream ~1.8x faster per row than K=64 on this silicon.

Scores are computed transposed: S^T quad [128 kcols, 4, 256 q] in a 2-bank
PSUM tile; ONE activation instruction computes exp(S/8) over the whole
quad (cross-bank read) into bf16 -- the ACT engine is the attention-phase
pacer, so exp instructions are batched to amortize its fixed overhead.
PV accumulates [128, 2, 65] in a single shared PSUM bank (one start=True
zeroes the whole bank; later groups accumulate onto zeros).

Projections use a fused [Wk|Wq] weight (M=128): K^T lands on partitions
0..63 (direct DVE copy), Q^T on 64..127 and hops to partitions 0..63 via a
small SBUF->SBUF DMA (stride-2 column select picks the parity rows).
Scheduling: the PE instruction stream is ordered by readiness -- next
quad's scores before the previous quad's PV, the next block's QK chain
drained right after quad 0 of each phase (so the Q-hop DMA completes long
before it's needed), and V-projection matmuls (64-row, cheap) popped as
filler between quads under a row budget to keep the PE from idling while
ACT streams exp.
"""

import os
import numpy as np
import ml_dtypes

B, T, E, HEAD, NH = 4, 4096, 1024, 64, 16
BLK = 512
NB = T // BLK  # 8 q/k blocks
KE = E // 128  # contraction chunks
NCORES = 8

_prog_cache = {}


def _build_program(reps=None, variant=None, popn=3):
    """variant (timing ablations only, breaks correctness):
    'nopv' = skip PV+normalize; 'noact' = also skip exp; 'proj' = projections
    and DMA only; 'dma' = input DMA only."""
    import concourse.bass as bass
    import concourse.mybir as mybir
    import concourse.tile as tile
    from concourse import bacc

    f32 = mybir.dt.float32
    bf16 = mybir.dt.bfloat16

    nc = bacc.Bacc("TRN2", target_bir_lowering=False, debug=False, num_devices=NCORES)

    f8 = mybir.dt.float8e4
    xT = nc.dram_tensor("xT", [E, T], f8, kind="ExternalInput").ap()
    xT0 = nc.dram_tensor("xT0", [E, BLK], bf16, kind="ExternalInput").ap()
    wkqb = nc.dram_tensor("wkqb", [E, 128], bf16, kind="ExternalInput").ap()
    wvb = nc.dram_tensor("wvb", [E, HEAD], bf16, kind="ExternalInput").ap()
    tri = nc.dram_tensor("tri", [128, 4, 256], bf16, kind="ExternalInput").ap()
    out = nc.dram_tensor("out", [128, 16, HEAD], f32, kind="ExternalOutput").ap()


    import contextlib

    POPN = popn

    with tile.TileContext(nc) as tc:
        loop_ctx = tc.For_i(0, reps, 1) if reps else contextlib.nullcontext()
        with (
            loop_ctx,
            tc.tile_pool(name="singles", bufs=1) as singles,
            tc.tile_pool(name="psum_proj", bufs=2, space="PSUM") as psum_proj,
            tc.tile_pool(name="psum_s", bufs=2, space="PSUM") as psum_s,
            tc.tile_pool(name="psum_o", bufs=2, space="PSUM") as psum_o,
            tc.tile_pool(name="ptil", bufs=4) as ptil_pool,
            tc.tile_pool(name="stage", bufs=2) as stage_pool,
            tc.tile_pool(name="work", bufs=4) as work,
        ):
            # ---- static inputs ----
            wkqb_sb = singles.tile([128, KE, 128], bf16)
            wvb_sb = singles.tile([128, KE, HEAD], bf16)
            tri_sb = singles.tile([128, 4, 256], bf16)
            nc.sync.dma_start(out=wkqb_sb, in_=wkqb.rearrange("(k p) d -> p k d", p=128))
            nc.sync.dma_start(out=wvb_sb, in_=wvb.rearrange("(k p) d -> p k d", p=128))
            nc.sync.dma_start(out=tri_sb, in_=tri)

            # ---- x, DMA'd per block in consumption order; block 0 bf16 ----
            x0_sb = singles.tile([128, KE, BLK], bf16)
            nc.sync.dma_start(out=x0_sb, in_=xT0.rearrange("(k p) t -> p k t", p=128))
            x_sb = singles.tile([128, KE, T], f8)
            xr = xT.rearrange("(k p) t -> p k t", p=128)
            for j in range(1, 4):
                eng = nc.scalar if j % 2 else nc.sync
                eng.dma_start(
                    out=x_sb[:, :, j * BLK:(j + 1) * BLK],
                    in_=xr[:, :, j * BLK:(j + 1) * BLK],
                )
            # tail blocks in one transfer: fewer DMAs amortize the ~1us+
            # per-DMA issue overhead; these blocks aren't needed until late
            nc.sync.dma_start(
                out=x_sb[:, :, 4 * BLK:], in_=xr[:, :, 4 * BLK:],
            )

            # K^T/Q^T padded to 128 contraction rows (upper 64 zeroed once,
            # on the otherwise-idle Pool engine): K=128 matmuls stream ~1.8x
            # faster per row than K=64 on this silicon.
            kt_sb = singles.tile([128, NB, BLK], bf16)    # K^T per block
            qt_sb = singles.tile([128, NB, 256], bf16)    # Q^T per block (own rows)
            nc.gpsimd.memset(kt_sb[64:128, :, :], 0.0)
            nc.gpsimd.memset(qt_sb[64:128, :, :], 0.0)
            v1_sb = singles.tile([128, NB, 4, HEAD + 1], bf16)  # V | ones
            nc.vector.memset(v1_sb[:, :, :, HEAD:HEAD + 1], 1.0)
            outs_sb = singles.tile([128, 16, HEAD], f32)
            if variant:
                nc.vector.memset(outs_sb, 0.0)

            # ---------- projection emitters (as thunk lists) ----------
            def qk_chain_thunks(j):
                """[Wk|Wq] @ x_block -> psum [128, 512]; then copies + Q hop."""
                ps = psum_proj.tile([128, BLK], f32, tag="proj", name=f"pqk{j}")

                def mk_mm(k):
                    def t():
                        rhs = (x0_sb[:, k, :] if j == 0
                               else x_sb[:, k, j * BLK:(j + 1) * BLK])
                        nc.tensor.matmul(
                            ps, wkqb_sb[:, k, :], rhs,
                            start=(k == 0), stop=(k == KE - 1),
                        )
                    return t

                def closer():
                    # K^T: partitions 0..63, straight copy
                    nc.vector.tensor_copy(kt_sb[0:64, j, :], ps[0:64, :])
                    # Q^T: partitions 64..127, parity-strided, hop to 0..63
                    st = stage_pool.tile([128, 256], bf16, tag="st")
                    nc.vector.tensor_copy(st[64:128, :], ps[64:128, 0:BLK:2])
                    nc.gpsimd.dma_start(out=qt_sb[0:64, j, :], in_=st[64:128, :])

                return [mk_mm(k) for k in range(KE)] + [closer]

            def v_chain_thunks(j):
                """x_block^T-stationary V projection -> one shared psum bank."""
                ps = psum_proj.tile([128, 4, HEAD], f32, tag="proj", name=f"pv{j}")

                def mk_mm(c, k):
                    def t():
                        lhsT = (x0_sb[:, k, c * 128:(c + 1) * 128] if j == 0
                                else x_sb[:, k,
                                          j * BLK + c * 128:j * BLK + (c + 1) * 128])
                        nc.tensor.matmul(
                            ps[:, c, :], lhsT, wvb_sb[:, k, :],
                            start=(c == 0 and k == 0),
                            stop=(c == 3 and k == KE - 1),
                            skip_group_check=True,
                        )
                    return t

                def closer():
                    nc.vector.tensor_copy(v1_sb[:, j, :, 0:HEAD], ps)

                return [mk_mm(c, k) for c in range(4) for k in range(KE)] + [closer]

            # thunk queue: proj work to interleave into attention phases.
            # FIFO of (key, rows, thunk); pop_rows(budget) emits thunks until
            # ~budget PE rows are issued, keeping the per-quad PE period
            # just under the ACT exp period. drain_key(k) emits everything
            # queued up to and including key k's thunks (dependency barrier).
            pending = []

            def pop_rows(budget):
                while pending and budget > 0:
                    _, rows, t = pending.pop(0)
                    t()
                    budget -= rows

            def pop_thunks(n):
                for _ in range(min(n, len(pending))):
                    pending.pop(0)[2]()

            def drain_key(key):
                while any(k == key for k, _, _ in pending):
                    pending.pop(0)[2]()

            # ---------- attention ----------
            def scores_quad(j, q):
                sq = psum_s.tile([128, 4, 256], f32, tag="s", name=f"s{j}_{q}")
                for c in range(4):
                    nc.tensor.matmul(
                        sq[:, c, :],
                        kt_sb[:, q, c * 128:(c + 1) * 128],
                        qt_sb[:, j, :],
                        start=(c % 2 == 0), stop=(c % 2 == 1),
                        skip_group_check=True,
                    )
                return sq

            def emit_phase(j):
                o_t = psum_o.tile([128, 2, HEAD + 1], f32, tag="o", name=f"o{j}")

                def pv(q, pt):
                    drain_key(("v", q))  # v1_sb[:, q] must be fully emitted
                    for c in range(4):
                        for g in range(2):
                            nc.tensor.matmul(
                                o_t[:, g, :],
                                pt[:, c, g * 128:(g + 1) * 128],
                                v1_sb[:, q, c, :],
                                start=(q == 0 and c == 0 and g == 0),
                                stop=(q == j and c == 3 and g == 1),
                                skip_group_check=True,
                            )

                drain_key(("qk", j))  # qt(j)/kt(j) must be fully emitted
                prev = None  # (q, pt) awaiting PV
                for q in range(0 if variant not in ("proj", "dma") else j + 1,
                               j + 1):
                    sq = scores_quad(j, q)
                    if variant not in ("noact",):
                        pt = ptil_pool.tile([128, 4, 256], bf16, tag="pt")
                        nc.scalar.activation(
                            pt, sq, mybir.ActivationFunctionType.Exp, scale=0.125
                        )
                        if q == j:
                            nc.vector.tensor_mul(pt, pt, tri_sb)
                        # filler BEFORE the blocking PV: the PE chews ready
                        # projection matmuls while ACT finishes exp(q-1), so
                        # it reaches PV(q-1) after its input is ready and
                        # never idles (idle gaps halve the PE clock).
                        if j < 2:
                            pop_thunks(12)
                        elif q == 0:
                            # next phase's QK chain + Q hop go out early so
                            # the hop DMA completes long before phase j+1
                            drain_key(("qk", j + 1))
                        else:
                            pop_rows(POPN)
                        if prev is not None and variant is None:
                            pv(*prev)
                        prev = (q, pt)
                    else:
                        pop_thunks(12)
                if variant is None:
                    pv(*prev)

                    # normalize: divide by the ones-column accumulation
                    recip = work.tile([128, 2, 1], f32, tag="recip")
                    nc.vector.reciprocal(recip, o_t[:, :, HEAD:HEAD + 1])
                    for g in range(2):
                        nc.vector.tensor_scalar_mul(
                            outs_sb[:, j * 2 + g, :], o_t[:, g, 0:HEAD],
                            recip[:, g, :]
                        )
                nc.gpsimd.dma_start(
                    out=out[:, j * 2:j * 2 + 2, :],
                    in_=outs_sb[:, j * 2:j * 2 + 2, :],
                )

            # ---------- main loop ----------
            # Queue order per phase j: [qk(j+1), v(j)] -- qk(j+1) (incl. the
            # Q hop DMA) pops early so phase j+1's first scores quad never
            # waits on the hop; v(j) is only needed by phase j's diagonal PV
            # (barrier drains it there at the latest). Pacing pops interleave
            # the projection matmuls into the PE stream during attention.
            if variant != "dma":
                pending.extend((("qk", 0), 512, t) for t in qk_chain_thunks(0))
            for j in range(NB):
                if variant != "dma":
                    if j + 1 < NB:
                        pending.extend(
                            (("qk", j + 1), 512, t) for t in qk_chain_thunks(j + 1))
                    pending.extend((("v", j), 64, t) for t in v_chain_thunks(j))
                emit_phase(j)
            pop_thunks(len(pending))

    nc.compile()
    return nc


def _host_inputs(embedded, Wq, Wk, Wv):
    """Per-core input maps (host does layout only: transpose/concat/cast)."""
    bf = ml_dtypes.bfloat16
    f8 = ml_dtypes.float8_e4m3
    emb = np.asarray(embedded, dtype=np.float32)
    wkqb = np.concatenate(
        [np.asarray(Wk, np.float32), np.asarray(Wq, np.float32)], axis=1
    ).astype(bf)
    wvb = np.asarray(Wv, dtype=np.float32).astype(bf)

    # The program always takes the EVEN columns of its x as "own" q-rows, so
    # parity-1 cores get x^T with each adjacent column pair swapped; the true
    # global position of local k-column m is then m ^ r, which the causal
    # mask (applied on the diagonal quad only) accounts for.
    # tri[p, c, f] = 1 if ((c*128 + p) ^ r) <= (2*f + r) else 0
    p = np.arange(128)[:, None, None]
    c = np.arange(4)[None, :, None]
    f = np.arange(256)[None, None, :]
    tris = [(((c * 128 + p) ^ r) <= (2 * f + r)).astype(bf) for r in range(2)]

    swap = np.arange(T) ^ 1  # pair-swap permutation
    in_maps = []
    for b in range(B):
        xTb = emb[b].T  # [E, T] f32
        xTs = [xTb, xTb[:, swap]]
        for r in range(2):
            in_maps.append({
                "xT": np.ascontiguousarray(xTs[r].astype(f8)),
                "xT0": np.ascontiguousarray(xTs[r][:, 0:BLK].astype(bf)),
                "wkqb": wkqb, "wvb": wvb,
                "tri": np.ascontiguousarray(tris[r]),
            })
    return in_maps


def _run(nc, in_maps, trace=False):
    from concourse.bass_utils import run_bass_kernel_spmd
    return run_bass_kernel_spmd(nc, in_maps, list(range(NCORES)), trace=trace)


def _assemble(results):
    head = np.empty((B, T, HEAD), dtype=np.float32)
    rows = np.arange(128)
    for core, res in enumerate(results):
        b, r = divmod(core, 2)
        o = np.asarray(res["out"])  # [128, 16, 64]
        for j in range(NB):
            for g in range(2):
                head[b, j * BLK + 2 * (g * 128 + rows) + r, :] = o[:, j * 2 + g, :]
    return np.tile(head, (1, 1, NH))


def kernel(embedded, Wq, Wk, Wv, num_heads):
    num_heads = int(num_heads)
    assert num_heads == NH

    if "nc" not in _prog_cache:
        _prog_cache["nc"] = _build_program()
    nc = _prog_cache["nc"]

    in_maps = _host_inputs(embedded, Wq, Wk, Wv)
    res = _run(nc, in_maps, trace=bool(int(os.environ.get("KERNEL_TRACE", "0"))))
    _prog_cache["last_result"] = res
    return _assemble(res.results)


# revision 35
# speedup vs baseline: 1.0368x; 1.0368x over previous
"""Causal single-head attention (shared-weight multi-head), 8-core Trainium2 Bass kernel.

Problem: embedded [4, 4096, 1024] f32, Wq/Wk/Wv [1024, 64] f32.
  q/k/v = embedded @ W*;  S = q k^T / 8 (causal);  P = softmax(S);  head = P v
  output = tile(head, 16) -> [4, 4096, 1024] f32.

Sharding: 8 cores = 4 batches x 2 parities. Core (b, r) owns q-rows
j*512 + 2*f + r for f in 0..255, j in 0..7 (every other row of every
512-block). Both cores of a batch then have IDENTICAL causal structure
(q-block j sees k-blocks 0..j) -- no padding, no role asymmetry. The host
pair-swaps x^T columns for parity-1 cores so "own" rows always sit at even
columns; the per-core triangular mask encodes the true k positions.

Inputs: x^T ships as fp8e4 except block 0, which stays bf16 -- rows with
few softmax candidates can't average out fp8 noise, every other row can
(measured). Weights stay bf16; mixed bf16 x fp8 matmuls are exact on HW.

Per q-block j: Q^T [64->128, 256] (own rows), K^T [64->128, 512] and
V [512, 65] per k-block (the 65th V column is ones so PV accumulates the
softmax denominator for free). Q^T/K^T are zero-padded to 128 contraction
rows (upper half zeroed once by the idle GPSIMD engine): K=128 matmuls
stream ~1.8x faster per row than K=64 on this silicon.

Scores are computed transposed: S^T quad [128 kcols, 4, 256 q] in a 2-bank
PSUM tile; ONE activation instruction computes exp(S/8) over the whole
quad (cross-bank read) into bf16 -- the ACT engine is the attention-phase
pacer, so exp instructions are batched to amortize its fixed overhead.
The diagonal quad only computes its causally-reachable ragged region
(widths 256/192/128/64 per k-chunk, packed bank-safely at offsets
0/256/512/640), trimming both PE rows and ACT elements by ~37% there.
PV accumulates [128, 2, 65] in a single shared PSUM bank (one start=True
zeroes the whole bank; later groups accumulate onto zeros).

Projections use a fused [Wk|Wq] weight (M=128): K^T lands on partitions
0..63 (direct DVE copy), Q^T on 64..127 and hops to partitions 0..63 via a
small SBUF->SBUF DMA (stride-2 column select picks the parity rows).
Scheduling: the PE instruction stream is ordered by readiness -- next
quad's scores before the previous quad's PV, the next block's QK chain
drained right after quad 0 of each phase (so the Q-hop DMA completes long
before it's needed), and V-projection matmuls (64-row, cheap) popped as
filler between quads under a row budget to keep the PE from idling while
ACT streams exp.
"""

import os
import numpy as np
import ml_dtypes

B, T, E, HEAD, NH = 4, 4096, 1024, 64, 16
BLK = 512
NB = T // BLK  # 8 q/k blocks
KE = E // 128  # contraction chunks
NCORES = 8

_prog_cache = {}


def _build_program(reps=None, variant=None, popn=3):
    """variant (timing ablations only, breaks correctness):
    'nopv' = skip PV+normalize; 'noact' = also skip exp; 'proj' = projections
    and DMA only; 'dma' = input DMA only."""
    import concourse.bass as bass
    import concourse.mybir as mybir
    import concourse.tile as tile
    from concourse import bacc

    f32 = mybir.dt.float32
    bf16 = mybir.dt.bfloat16

    nc = bacc.Bacc("TRN2", target_bir_lowering=False, debug=False, num_devices=NCORES)

    f8 = mybir.dt.float8e4
    xT = nc.dram_tensor("xT", [E, T], f8, kind="ExternalInput").ap()
    xT0 = nc.dram_tensor("xT0", [E, BLK], bf16, kind="ExternalInput").ap()
    wkqb = nc.dram_tensor("wkqb", [E, 128], bf16, kind="ExternalInput").ap()
    wvb = nc.dram_tensor("wvb", [E, HEAD], bf16, kind="ExternalInput").ap()
    tri = nc.dram_tensor("tri", [128, 704], bf16, kind="ExternalInput").ap()
    out = nc.dram_tensor("out", [128, 16, HEAD], f32, kind="ExternalOutput").ap()


    import contextlib

    POPN = popn

    with tile.TileContext(nc) as tc:
        loop_ctx = tc.For_i(0, reps, 1) if reps else contextlib.nullcontext()
        with (
            loop_ctx,
            tc.tile_pool(name="singles", bufs=1) as singles,
            tc.tile_pool(name="psum_proj", bufs=2, space="PSUM") as psum_proj,
            tc.tile_pool(name="psum_s", bufs=2, space="PSUM") as psum_s,
            tc.tile_pool(name="psum_o", bufs=2, space="PSUM") as psum_o,
            tc.tile_pool(name="ptil", bufs=4) as ptil_pool,
            tc.tile_pool(name="stage", bufs=2) as stage_pool,
            tc.tile_pool(name="work", bufs=4) as work,
        ):
            # ---- static inputs ----
            wkqb_sb = singles.tile([128, KE, 128], bf16)
            wvb_sb = singles.tile([128, KE, HEAD], bf16)
            tri_sb = singles.tile([128, 704], bf16)

            # ---- x, DMA'd per block in consumption order; block 0 bf16.
            # x0 gates the whole pipeline: it goes FIRST on the sync queue,
            # split in two halves; weights/tri go on the ACT queue so they
            # transfer concurrently rather than ahead of x0. ----
            x0_sb = singles.tile([128, KE, BLK], bf16)
            x0r = xT0.rearrange("(k p) t -> p k t", p=128)
            nc.sync.dma_start(out=x0_sb[:, 0:KE // 2, :], in_=x0r[:, 0:KE // 2, :])
            nc.sync.dma_start(out=x0_sb[:, KE // 2:, :], in_=x0r[:, KE // 2:, :])
            nc.scalar.dma_start(out=wkqb_sb, in_=wkqb.rearrange("(k p) d -> p k d", p=128))
            nc.scalar.dma_start(out=wvb_sb, in_=wvb.rearrange("(k p) d -> p k d", p=128))
            nc.scalar.dma_start(out=tri_sb, in_=tri)
            x_sb = singles.tile([128, KE, T], f8)
            xr = xT.rearrange("(k p) t -> p k t", p=128)
            for j in range(1, 4):
                eng = nc.scalar if j % 2 else nc.sync
                eng.dma_start(
                    out=x_sb[:, :, j * BLK:(j + 1) * BLK],
                    in_=xr[:, :, j * BLK:(j + 1) * BLK],
                )
            # tail blocks in one transfer: fewer DMAs amortize the ~1us+
            # per-DMA issue overhead; these blocks aren't needed until late
            nc.sync.dma_start(
                out=x_sb[:, :, 4 * BLK:], in_=xr[:, :, 4 * BLK:],
            )

            # K^T/Q^T padded to 128 contraction rows (upper 64 zeroed once,
            # on the otherwise-idle Pool engine): K=128 matmuls stream ~1.8x
            # faster per row than K=64 on this silicon.
            kt_sb = singles.tile([128, NB, BLK], bf16)    # K^T per block
            qt_sb = singles.tile([128, NB, 256], bf16)    # Q^T per block (own rows)
            nc.gpsimd.memset(kt_sb[64:128, :, :], 0.0)
            nc.gpsimd.memset(qt_sb[64:128, :, :], 0.0)
            v1_sb = singles.tile([128, NB, 4, HEAD + 1], bf16)  # V | ones
            nc.vector.memset(v1_sb[:, :, :, HEAD:HEAD + 1], 1.0)
            outs_sb = singles.tile([128, 16, HEAD], f32)
            if variant:
                nc.vector.memset(outs_sb, 0.0)

            # ---------- projection emitters (as thunk lists) ----------
            def qk_chain_thunks(j):
                """[Wk|Wq] @ x_block -> psum [128, 512]; then copies + Q hop."""
                ps = psum_proj.tile([128, BLK], f32, tag="proj", name=f"pqk{j}")

                def mk_mm(k):
                    def t():
                        rhs = (x0_sb[:, k, :] if j == 0
                               else x_sb[:, k, j * BLK:(j + 1) * BLK])
                        nc.tensor.matmul(
                            ps, wkqb_sb[:, k, :], rhs,
                            start=(k == 0), stop=(k == KE - 1),
                        )
                    return t

                def closer():
                    # K^T: partitions 0..63, straight copy
                    nc.vector.tensor_copy(kt_sb[0:64, j, :], ps[0:64, :])
                    # Q^T: partitions 64..127, parity-strided, hop to 0..63
                    st = stage_pool.tile([128, 256], bf16, tag="st")
                    nc.vector.tensor_copy(st[64:128, :], ps[64:128, 0:BLK:2])
                    nc.gpsimd.dma_start(out=qt_sb[0:64, j, :], in_=st[64:128, :])

                return [mk_mm(k) for k in range(KE)] + [closer]

            def v_chain_thunks(j):
                """x_block^T-stationary V projection -> one shared psum bank."""
                ps = psum_proj.tile([128, 4, HEAD], f32, tag="proj", name=f"pv{j}")

                def mk_mm(c, k):
                    def t():
                        lhsT = (x0_sb[:, k, c * 128:(c + 1) * 128] if j == 0
                                else x_sb[:, k,
                                          j * BLK + c * 128:j * BLK + (c + 1) * 128])
                        nc.tensor.matmul(
                            ps[:, c, :], lhsT, wvb_sb[:, k, :],
                            start=(c == 0 and k == 0),
                            stop=(c == 3 and k == KE - 1),
                            skip_group_check=True,
                        )
                    return t

                def closer():
                    nc.vector.tensor_copy(v1_sb[:, j, :, 0:HEAD], ps)

                return [mk_mm(c, k) for c in range(4) for k in range(KE)] + [closer]

            # thunk queue: proj work to interleave into attention phases.
            # FIFO of (key, rows, thunk); pop_rows(budget) emits thunks until
            # ~budget PE rows are issued, keeping the per-quad PE period
            # just under the ACT exp period. drain_key(k) emits everything
            # queued up to and including key k's thunks (dependency barrier).
            pending = []

            def pop_rows(budget):
                while pending and budget > 0:
                    _, rows, t = pending.pop(0)
                    t()
                    budget -= rows

            def pop_thunks(n):
                for _ in range(min(n, len(pending))):
                    pending.pop(0)[2]()

            def drain_key(key):
                while any(k == key for k, _, _ in pending):
                    pending.pop(0)[2]()

            # ---------- attention ----------
            DOFF = [0, 256, 512, 640]   # packed diag offsets (bank-safe)
            DW = [256, 192, 128, 64]    # widths: only f >= 64c is unmasked

            def scores_quad(j, q):
                if q == j:
                    sq = psum_s.tile([128, 704], f32, tag="s", name=f"sd{j}")
                    for c in range(4):
                        nc.tensor.matmul(
                            sq[:, DOFF[c]:DOFF[c] + DW[c]],
                            kt_sb[:, q, c * 128:(c + 1) * 128],
                            qt_sb[:, j, 64 * c:256],
                            start=(c in (0, 2)), stop=(c in (1, 3)),
                            skip_group_check=True,
                        )
                    return sq
                sq = psum_s.tile([128, 4, 256], f32, tag="s", name=f"s{j}_{q}")
                for c in range(4):
                    nc.tensor.matmul(
                        sq[:, c, :],
                        kt_sb[:, q, c * 128:(c + 1) * 128],
                        qt_sb[:, j, :],
                        start=(c % 2 == 0), stop=(c % 2 == 1),
                        skip_group_check=True,
                    )
                return sq

            def emit_phase(j):
                o_t = psum_o.tile([128, 2, HEAD + 1], f32, tag="o", name=f"o{j}")

                def pv(q, pt):
                    drain_key(("v", q))  # v1_sb[:, q] must be fully emitted
                    if q == j:
                        # packed ragged diag: chunk c covers f in [64c, 256);
                        # emit per (c, q-group) overlap, offset into o_t's
                        # partitions
                        first = (q == 0)
                        segs = []
                        for c in range(4):
                            for g in range(2):
                                lo = max(64 * c, 128 * g)
                                hi = min(256, 128 * g + 128)
                                if lo < hi:
                                    segs.append((c, g, lo, hi))
                        for i, (c, g, lo, hi) in enumerate(segs):
                            nc.tensor.matmul(
                                o_t[lo - 128 * g:hi - 128 * g, g, :],
                                pt[:, DOFF[c] + lo - 64 * c:
                                   DOFF[c] + hi - 64 * c],
                                v1_sb[:, q, c, :],
                                start=(first and i == 0),
                                stop=(i == len(segs) - 1),
                                skip_group_check=True,
                            )
                        return
                    for c in range(4):
                        for g in range(2):
                            nc.tensor.matmul(
                                o_t[:, g, :],
                                pt[:, c, g * 128:(g + 1) * 128],
                                v1_sb[:, q, c, :],
                                start=(q == 0 and c == 0 and g == 0),
                                stop=False,
                                skip_group_check=True,
                            )

                drain_key(("qk", j))  # qt(j)/kt(j) must be fully emitted
                prev = None  # (q, pt) awaiting PV
                for q in range(0 if variant not in ("proj", "dma") else j + 1,
                               j + 1):
                    sq = scores_quad(j, q)
                    if variant not in ("noact",):
                        pt = ptil_pool.tile(
                            [128, 704] if q == j else [128, 4, 256],
                            bf16, tag="pt")
                        nc.scalar.activation(
                            pt, sq, mybir.ActivationFunctionType.Exp, scale=0.125
                        )
                        if q == j:
                            nc.vector.tensor_mul(pt, pt, tri_sb)
                        # filler BEFORE the blocking PV: the PE chews ready
                        # projection matmuls while ACT finishes exp(q-1), so
                        # it reaches PV(q-1) after its input is ready and
                        # never idles (idle gaps halve the PE clock).
                        if j < 2:
                            pop_thunks(12)
                        elif q == 0:
                            # next phase's QK chain + Q hop go out early so
                            # the hop DMA completes long before phase j+1
                            drain_key(("qk", j + 1))
                        else:
                            pop_rows(POPN)
                        if prev is not None and variant is None:
                            pv(*prev)
                        prev = (q, pt)
                    else:
                        pop_thunks(12)
                if variant is None:
                    pv(*prev)

                    # normalize: divide by the ones-column accumulation
                    recip = work.tile([128, 2, 1], f32, tag="recip")
                    nc.vector.reciprocal(recip, o_t[:, :, HEAD:HEAD + 1])
                    for g in range(2):
                        nc.vector.tensor_scalar_mul(
                            outs_sb[:, j * 2 + g, :], o_t[:, g, 0:HEAD],
                            recip[:, g, :]
                        )
                nc.gpsimd.dma_start(
                    out=out[:, j * 2:j * 2 + 2, :],
                    in_=outs_sb[:, j * 2:j * 2 + 2, :],
                )

            # ---------- main loop ----------
            # Queue order per phase j: [qk(j+1), v(j)] -- qk(j+1) (incl. the
            # Q hop DMA) pops early so phase j+1's first scores quad never
            # waits on the hop; v(j) is only needed by phase j's diagonal PV
            # (barrier drains it there at the latest). Pacing pops interleave
            # the projection matmuls into the PE stream during attention.
            if variant != "dma":
                pending.extend((("qk", 0), 512, t) for t in qk_chain_thunks(0))
            for j in range(NB):
                if variant != "dma":
                    if j + 1 < NB:
                        pending.extend(
                            (("qk", j + 1), 512, t) for t in qk_chain_thunks(j + 1))
                    pending.extend((("v", j), 64, t) for t in v_chain_thunks(j))
                emit_phase(j)
            pop_thunks(len(pending))

    nc.compile()
    return nc


def _host_inputs(embedded, Wq, Wk, Wv):
    """Per-core input maps (host does layout only: transpose/concat/cast)."""
    bf = ml_dtypes.bfloat16
    f8 = ml_dtypes.float8_e4m3
    emb = np.asarray(embedded, dtype=np.float32)
    wkqb = np.concatenate(
        [np.asarray(Wk, np.float32), np.asarray(Wq, np.float32)], axis=1
    ).astype(bf)
    wvb = np.asarray(Wv, dtype=np.float32).astype(bf)

    # The program always takes the EVEN columns of its x as "own" q-rows, so
    # parity-1 cores get x^T with each adjacent column pair swapped; the true
    # global position of local k-column m is then m ^ r, which the causal
    # mask (applied on the diagonal quad only) accounts for.
    # Packed ragged diag layout: chunk c holds q-rows f in [64c, 256) at
    # column DOFF[c] + f - 64c; mask = ((c*128 + p) ^ r) <= (2*f + r).
    DOFF = [0, 256, 512, 640]
    tris = []
    for r in range(2):
        t = np.zeros((128, 704), np.float32)
        p = np.arange(128)[:, None]
        for c in range(4):
            f = np.arange(64 * c, 256)[None, :]
            t[:, DOFF[c]:DOFF[c] + 256 - 64 * c] = (
                ((c * 128 + p) ^ r) <= (2 * f + r))
        tris.append(t.astype(bf))

    swap = np.arange(T) ^ 1  # pair-swap permutation
    in_maps = []
    for b in range(B):
        xTb = emb[b].T  # [E, T] f32
        xTs = [xTb, xTb[:, swap]]
        for r in range(2):
            in_maps.append({
                "xT": np.ascontiguousarray(xTs[r].astype(f8)),
                "xT0": np.ascontiguousarray(xTs[r][:, 0:BLK].astype(bf)),
                "wkqb": wkqb, "wvb": wvb,
                "tri": np.ascontiguousarray(tris[r]),
            })
    return in_maps


def _run(nc, in_maps, trace=False):
    from concourse.bass_utils import run_bass_kernel_spmd
    return run_bass_kernel_spmd(nc, in_maps, list(range(NCORES)), trace=trace)


def _assemble(results):
    head = np.empty((B, T, HEAD), dtype=np.float32)
    rows = np.arange(128)
    for core, res in enumerate(results):
        b, r = divmod(core, 2)
        o = np.asarray(res["out"])  # [128, 16, 64]
        for j in range(NB):
            for g in range(2):
                head[b, j * BLK + 2 * (g * 128 + rows) + r, :] = o[:, j * 2 + g, :]
    return np.tile(head, (1, 1, NH))


def kernel(embedded, Wq, Wk, Wv, num_heads):
    num_heads = int(num_heads)
    assert num_heads == NH

    if "nc" not in _prog_cache:
        _prog_cache["nc"] = _build_program()
    nc = _prog_cache["nc"]

    in_maps = _host_inputs(embedded, Wq, Wk, Wv)
    res = _run(nc, in_maps, trace=bool(int(os.environ.get("KERNEL_TRACE", "0"))))
    _prog_cache["last_result"] = res
    return _assemble(res.results)


# revision 36
# speedup vs baseline: 1.0916x; 1.0529x over previous
"""Causal single-head attention (shared-weight multi-head), 8-core Trainium2 Bass kernel.

Problem: embedded [4, 4096, 1024] f32, Wq/Wk/Wv [1024, 64] f32.
  q/k/v = embedded @ W*;  S = q k^T / 8 (causal);  P = softmax(S);  head = P v
  output = tile(head, 16) -> [4, 4096, 1024] f32.

Sharding: 8 cores = 4 batches x 2 parities. Core (b, r) owns q-rows
j*512 + 2*f + r for f in 0..255, j in 0..7 (every other row of every
512-block). Both cores of a batch then have IDENTICAL causal structure
(q-block j sees k-blocks 0..j) -- no padding, no role asymmetry. The host
pair-swaps x^T columns for parity-1 cores so "own" rows always sit at even
columns; the per-core triangular mask encodes the true k positions.

Inputs: x^T ships as fp8e4 except block 0, which stays bf16 -- rows with
few softmax candidates can't average out fp8 noise, every other row can
(measured). Weights stay bf16; mixed bf16 x fp8 matmuls are exact on HW.

Per q-block j: Q^T [64->128, 256] (own rows), K^T [64->128, 512] and
V [512, 65] per k-block (the 65th V column is ones so PV accumulates the
softmax denominator for free). Q^T/K^T are zero-padded to 128 contraction
rows (upper half zeroed once by the idle GPSIMD engine): K=128 matmuls
stream ~1.8x faster per row than K=64 on this silicon.

Scores are computed transposed: S^T quad [128 kcols, 4, 256 q] in a 2-bank
PSUM tile; ONE activation instruction computes exp(S/8) over the whole
quad (cross-bank read) into bf16 -- the ACT engine is the attention-phase
pacer, so exp instructions are batched to amortize its fixed overhead.
The diagonal quad only computes its causally-reachable ragged region
(widths 256/192/128/64 per k-chunk, packed bank-safely at offsets
0/256/512/640), trimming both PE rows and ACT elements by ~37% there.
PV accumulates [128, 2, 65] in a single shared PSUM bank (one start=True
zeroes the whole bank; later groups accumulate onto zeros).

Projections use a fused [Wk|Wq] weight (M=128): K^T lands on partitions
0..63 (direct DVE copy), Q^T on 64..127 and hops to partitions 0..63 via a
small SBUF->SBUF DMA (stride-2 column select picks the parity rows).
Scheduling: the PE instruction stream is ordered by readiness -- next
quad's scores before the previous quad's PV, the next block's QK chain
drained right after quad 0 of each phase (so the Q-hop DMA completes long
before it's needed), and V-projection matmuls (64-row, cheap) popped as
filler between quads under a row budget to keep the PE from idling while
ACT streams exp.
"""

import os
import numpy as np
import ml_dtypes

B, T, E, HEAD, NH = 4, 4096, 1024, 64, 16
BLK = 512
NB = T // BLK  # 8 q/k blocks
KE = E // 128  # contraction chunks
NCORES = 8

_prog_cache = {}


def _build_program(reps=None, variant=None, popn=3):
    """variant (timing ablations only, breaks correctness):
    'nopv' = skip PV+normalize; 'noact' = also skip exp; 'proj' = projections
    and DMA only; 'dma' = input DMA only."""
    import concourse.bass as bass
    import concourse.mybir as mybir
    import concourse.tile as tile
    from concourse import bacc

    f32 = mybir.dt.float32
    bf16 = mybir.dt.bfloat16

    nc = bacc.Bacc("TRN2", target_bir_lowering=False, debug=False, num_devices=NCORES)

    f8 = mybir.dt.float8e4
    xT = nc.dram_tensor("xT", [E, T], f8, kind="ExternalInput").ap()
    xT0 = nc.dram_tensor("xT0", [E, BLK], bf16, kind="ExternalInput").ap()
    wkqb = nc.dram_tensor("wkqb", [E, 128], bf16, kind="ExternalInput").ap()
    wvb = nc.dram_tensor("wvb", [E, HEAD], bf16, kind="ExternalInput").ap()
    tri = nc.dram_tensor("tri", [128, 704], bf16, kind="ExternalInput").ap()
    out = nc.dram_tensor("out", [128, 16, HEAD], f32, kind="ExternalOutput").ap()


    import contextlib

    POPN = popn

    with tile.TileContext(nc) as tc:
        loop_ctx = tc.For_i(0, reps, 1) if reps else contextlib.nullcontext()
        with (
            loop_ctx,
            tc.tile_pool(name="singles", bufs=1) as singles,
            tc.tile_pool(name="psum_proj", bufs=2, space="PSUM") as psum_proj,
            tc.tile_pool(name="psum_s", bufs=2, space="PSUM") as psum_s,
            tc.tile_pool(name="psum_o", bufs=2, space="PSUM") as psum_o,
            tc.tile_pool(name="ptil", bufs=6) as ptil_pool,
            tc.tile_pool(name="stage", bufs=2) as stage_pool,
            tc.tile_pool(name="work", bufs=4) as work,
        ):
            # ---- static inputs ----
            wkqb_sb = singles.tile([128, KE, 128], bf16)
            wvb_sb = singles.tile([128, KE, HEAD], bf16)
            tri_sb = singles.tile([128, 704], bf16)

            # ---- x, DMA'd per block in consumption order; block 0 bf16.
            # x0 gates the whole pipeline: it goes FIRST on the sync queue,
            # split in two halves; weights/tri go on the ACT queue so they
            # transfer concurrently rather than ahead of x0. ----
            x0_sb = singles.tile([128, KE, BLK], bf16)
            x0r = xT0.rearrange("(k p) t -> p k t", p=128)
            nc.sync.dma_start(out=x0_sb[:, 0:KE // 2, :], in_=x0r[:, 0:KE // 2, :])
            nc.sync.dma_start(out=x0_sb[:, KE // 2:, :], in_=x0r[:, KE // 2:, :])
            nc.scalar.dma_start(out=wkqb_sb, in_=wkqb.rearrange("(k p) d -> p k d", p=128))
            nc.scalar.dma_start(out=wvb_sb, in_=wvb.rearrange("(k p) d -> p k d", p=128))
            nc.scalar.dma_start(out=tri_sb, in_=tri)
            x_sb = singles.tile([128, KE, T], f8)
            xr = xT.rearrange("(k p) t -> p k t", p=128)
            for j in range(1, 4):
                eng = nc.scalar if j % 2 else nc.sync
                eng.dma_start(
                    out=x_sb[:, :, j * BLK:(j + 1) * BLK],
                    in_=xr[:, :, j * BLK:(j + 1) * BLK],
                )
            # tail blocks in one transfer: fewer DMAs amortize the ~1us+
            # per-DMA issue overhead; these blocks aren't needed until late
            nc.sync.dma_start(
                out=x_sb[:, :, 4 * BLK:], in_=xr[:, :, 4 * BLK:],
            )

            # K^T/Q^T padded to 128 contraction rows (upper 64 zeroed once,
            # on the otherwise-idle Pool engine): K=128 matmuls stream ~1.8x
            # faster per row than K=64 on this silicon.
            kt_sb = singles.tile([128, NB, BLK], bf16)    # K^T per block
            qt_sb = singles.tile([128, NB, 256], bf16)    # Q^T per block (own rows)
            nc.gpsimd.memset(kt_sb[64:128, :, :], 0.0)
            nc.gpsimd.memset(qt_sb[64:128, :, :], 0.0)
            v1_sb = singles.tile([128, NB, 4, HEAD + 1], bf16)  # V | ones
            nc.vector.memset(v1_sb[:, :, :, HEAD:HEAD + 1], 1.0)
            outs_sb = singles.tile([128, 16, HEAD], f32)
            if variant:
                nc.vector.memset(outs_sb, 0.0)

            # ---------- projection emitters (as thunk lists) ----------
            def qk_chain_thunks(j):
                """[Wk|Wq] @ x_block -> psum [128, 512]; then copies + Q hop."""
                ps = psum_proj.tile([128, BLK], f32, tag="proj", name=f"pqk{j}")

                def mk_mm(k):
                    def t():
                        rhs = (x0_sb[:, k, :] if j == 0
                               else x_sb[:, k, j * BLK:(j + 1) * BLK])
                        nc.tensor.matmul(
                            ps, wkqb_sb[:, k, :], rhs,
                            start=(k == 0), stop=(k == KE - 1),
                        )
                    return t

                def closer():
                    # K^T: partitions 0..63, straight copy
                    nc.vector.tensor_copy(kt_sb[0:64, j, :], ps[0:64, :])
                    # Q^T: partitions 64..127, parity-strided, hop to 0..63
                    st = stage_pool.tile([128, 256], bf16, tag="st")
                    nc.vector.tensor_copy(st[64:128, :], ps[64:128, 0:BLK:2])
                    nc.gpsimd.dma_start(out=qt_sb[0:64, j, :], in_=st[64:128, :])

                return [mk_mm(k) for k in range(KE)] + [closer]

            def v_chain_thunks(j):
                """x_block^T-stationary V projection -> one shared psum bank."""
                ps = psum_proj.tile([128, 4, HEAD], f32, tag="proj", name=f"pv{j}")

                def mk_mm(c, k):
                    def t():
                        lhsT = (x0_sb[:, k, c * 128:(c + 1) * 128] if j == 0
                                else x_sb[:, k,
                                          j * BLK + c * 128:j * BLK + (c + 1) * 128])
                        nc.tensor.matmul(
                            ps[:, c, :], lhsT, wvb_sb[:, k, :],
                            start=(c == 0 and k == 0),
                            stop=(c == 3 and k == KE - 1),
                            skip_group_check=True,
                        )
                    return t

                def closer():
                    nc.vector.tensor_copy(v1_sb[:, j, :, 0:HEAD], ps)

                return [mk_mm(c, k) for c in range(4) for k in range(KE)] + [closer]

            # thunk queue: proj work to interleave into attention phases.
            # FIFO of (key, rows, thunk); pop_rows(budget) emits thunks until
            # ~budget PE rows are issued, keeping the per-quad PE period
            # just under the ACT exp period. drain_key(k) emits everything
            # queued up to and including key k's thunks (dependency barrier).
            pending = []

            def pop_rows(budget):
                while pending and budget > 0:
                    _, rows, t = pending.pop(0)
                    t()
                    budget -= rows

            def pop_thunks(n):
                for _ in range(min(n, len(pending))):
                    pending.pop(0)[2]()

            def drain_key(key):
                while any(k == key for k, _, _ in pending):
                    pending.pop(0)[2]()

            # ---------- attention ----------
            DOFF = [0, 256, 512, 640]   # packed diag offsets (bank-safe)
            DW = [256, 192, 128, 64]    # widths: only f >= 64c is unmasked

            def scores_quad(j, q):
                if q == j:
                    sq = psum_s.tile([128, 704], f32, tag="s", name=f"sd{j}")
                    for c in range(4):
                        nc.tensor.matmul(
                            sq[:, DOFF[c]:DOFF[c] + DW[c]],
                            kt_sb[:, q, c * 128:(c + 1) * 128],
                            qt_sb[:, j, 64 * c:256],
                            start=(c in (0, 2)), stop=(c in (1, 3)),
                            skip_group_check=True,
                        )
                    return sq
                sq = psum_s.tile([128, 4, 256], f32, tag="s", name=f"s{j}_{q}")
                for c in range(4):
                    nc.tensor.matmul(
                        sq[:, c, :],
                        kt_sb[:, q, c * 128:(c + 1) * 128],
                        qt_sb[:, j, :],
                        start=(c % 2 == 0), stop=(c % 2 == 1),
                        skip_group_check=True,
                    )
                return sq

            def emit_phase(j):
                o_t = psum_o.tile([128, 2, HEAD + 1], f32, tag="o", name=f"o{j}")

                def pv(q, pt):
                    drain_key(("v", q))  # v1_sb[:, q] must be fully emitted
                    if q == j:
                        # packed ragged diag: chunk c covers f in [64c, 256);
                        # emit per (c, q-group) overlap, offset into o_t's
                        # partitions
                        first = (q == 0)
                        segs = []
                        for c in range(4):
                            for g in range(2):
                                lo = max(64 * c, 128 * g)
                                hi = min(256, 128 * g + 128)
                                if lo < hi:
                                    segs.append((c, g, lo, hi))
                        for i, (c, g, lo, hi) in enumerate(segs):
                            nc.tensor.matmul(
                                o_t[lo - 128 * g:hi - 128 * g, g, :],
                                pt[:, DOFF[c] + lo - 64 * c:
                                   DOFF[c] + hi - 64 * c],
                                v1_sb[:, q, c, :],
                                start=(first and i == 0),
                                stop=(i == len(segs) - 1),
                                skip_group_check=True,
                            )
                        return
                    for c in range(4):
                        for g in range(2):
                            nc.tensor.matmul(
                                o_t[:, g, :],
                                pt[:, c, g * 128:(g + 1) * 128],
                                v1_sb[:, q, c, :],
                                start=(q == 0 and c == 0 and g == 0),
                                stop=False,
                                skip_group_check=True,
                            )

                drain_key(("qk", j))  # qt(j)/kt(j) must be fully emitted
                prev = None  # (q, pt) awaiting PV
                for q in range(0 if variant not in ("proj", "dma") else j + 1,
                               j + 1):
                    sq = scores_quad(j, q)
                    if variant not in ("noact",):
                        pt = ptil_pool.tile(
                            [128, 704] if q == j else [128, 4, 256],
                            bf16, tag="pt")
                        nc.scalar.activation(
                            pt, sq, mybir.ActivationFunctionType.Exp, scale=0.125
                        )
                        if q == j:
                            nc.vector.tensor_mul(pt, pt, tri_sb)
                        # filler BEFORE the blocking PV: the PE chews ready
                        # projection matmuls while ACT finishes exp(q-1), so
                        # it reaches PV(q-1) after its input is ready and
                        # never idles (idle gaps halve the PE clock).
                        if j < 2:
                            pop_thunks(12)
                        elif q == 0:
                            # next phase's QK chain + Q hop go out early so
                            # the hop DMA completes long before phase j+1
                            drain_key(("qk", j + 1))
                        else:
                            pop_rows(POPN)
                        if prev is not None and variant is None:
                            pv(*prev)
                        prev = (q, pt)
                    else:
                        pop_thunks(12)
                if variant is None:
                    pv(*prev)

                    # normalize: divide by the ones-column accumulation
                    recip = work.tile([128, 2, 1], f32, tag="recip")
                    nc.vector.reciprocal(recip, o_t[:, :, HEAD:HEAD + 1])
                    for g in range(2):
                        nc.vector.tensor_scalar_mul(
                            outs_sb[:, j * 2 + g, :], o_t[:, g, 0:HEAD],
                            recip[:, g, :]
                        )
                nc.gpsimd.dma_start(
                    out=out[:, j * 2:j * 2 + 2, :],
                    in_=outs_sb[:, j * 2:j * 2 + 2, :],
                )

            # ---------- main loop ----------
            # Queue order per phase j: [qk(j+1), v(j)] -- qk(j+1) (incl. the
            # Q hop DMA) pops early so phase j+1's first scores quad never
            # waits on the hop; v(j) is only needed by phase j's diagonal PV
            # (barrier drains it there at the latest). Pacing pops interleave
            # the projection matmuls into the PE stream during attention.
            if variant != "dma":
                pending.extend((("qk", 0), 512, t) for t in qk_chain_thunks(0))
            for j in range(NB):
                if variant != "dma":
                    if j + 1 < NB:
                        pending.extend(
                            (("qk", j + 1), 512, t) for t in qk_chain_thunks(j + 1))
                    pending.extend((("v", j), 64, t) for t in v_chain_thunks(j))
                emit_phase(j)
            pop_thunks(len(pending))

    nc.compile()
    return nc


def _host_inputs(embedded, Wq, Wk, Wv):
    """Per-core input maps (host does layout only: transpose/concat/cast)."""
    bf = ml_dtypes.bfloat16
    f8 = ml_dtypes.float8_e4m3
    emb = np.asarray(embedded, dtype=np.float32)
    wkqb = np.concatenate(
        [np.asarray(Wk, np.float32), np.asarray(Wq, np.float32)], axis=1
    ).astype(bf)
    wvb = np.asarray(Wv, dtype=np.float32).astype(bf)

    # The program always takes the EVEN columns of its x as "own" q-rows, so
    # parity-1 cores get x^T with each adjacent column pair swapped; the true
    # global position of local k-column m is then m ^ r, which the causal
    # mask (applied on the diagonal quad only) accounts for.
    # Packed ragged diag layout: chunk c holds q-rows f in [64c, 256) at
    # column DOFF[c] + f - 64c; mask = ((c*128 + p) ^ r) <= (2*f + r).
    DOFF = [0, 256, 512, 640]
    tris = []
    for r in range(2):
        t = np.zeros((128, 704), np.float32)
        p = np.arange(128)[:, None]
        for c in range(4):
            f = np.arange(64 * c, 256)[None, :]
            t[:, DOFF[c]:DOFF[c] + 256 - 64 * c] = (
                ((c * 128 + p) ^ r) <= (2 * f + r))
        tris.append(t.astype(bf))

    swap = np.arange(T) ^ 1  # pair-swap permutation
    in_maps = []
    for b in range(B):
        xTb = emb[b].T  # [E, T] f32
        xTs = [xTb, xTb[:, swap]]
        for r in range(2):
            in_maps.append({
                "xT": np.ascontiguousarray(xTs[r].astype(f8)),
                "xT0": np.ascontiguousarray(xTs[r][:, 0:BLK].astype(bf)),
                "wkqb": wkqb, "wvb": wvb,
                "tri": np.ascontiguousarray(tris[r]),
            })
    return in_maps


def _run(nc, in_maps, trace=False):
    from concourse.bass_utils import run_bass_kernel_spmd
    return run_bass_kernel_spmd(nc, in_maps, list(range(NCORES)), trace=trace)


def _assemble(results):
    head = np.empty((B, T, HEAD), dtype=np.float32)
    rows = np.arange(128)
    for core, res in enumerate(results):
        b, r = divmod(core, 2)
        o = np.asarray(res["out"])  # [128, 16, 64]
        for j in range(NB):
            for g in range(2):
                head[b, j * BLK + 2 * (g * 128 + rows) + r, :] = o[:, j * 2 + g, :]
    return np.tile(head, (1, 1, NH))


def kernel(embedded, Wq, Wk, Wv, num_heads):
    num_heads = int(num_heads)
    assert num_heads == NH

    if "nc" not in _prog_cache:
        _prog_cache["nc"] = _build_program()
    nc = _prog_cache["nc"]

    in_maps = _host_inputs(embedded, Wq, Wk, Wv)
    res = _run(nc, in_maps, trace=bool(int(os.environ.get("KERNEL_TRACE", "0"))))
    _prog_cache["last_result"] = res
    return _assemble(res.results)
